# revision 5
# baseline (speedup 1.0000x reference)
"""Trainium2 Bass kernel for nn_DenseSparsePreEmbedding.

Math refactoring (verified bit-exact vs the jax reference on CPU):
    fixed_emb @ W_fixed  == (fixed_table @ W_fixed)[fixed_features]
    sparse_emb @ W_sparse== (concat(tabs) @ W_sparse)[cv]  with cv the
                            combined per-token sparse code (last write wins,
                            sentinel 256 -> zero row for untouched tokens)
so the whole module collapses to a dual embedding gather + add:
    out[n] = tabA[ffn] + tabB[cvn]
with tabA = fixed_table @ W_fixed + b   [2048, 128]
     tabB = concat(tab0..3) @ W_sparse (+ zero row)  [257, 128]

Device kernel ("packed ap_gather", SPMD over 8 cores, 125000 tokens each):
  Both tables live in SBUF as ONE [128, 2048] uint32 image of packed bf16
  pairs: partitions 0-63 hold tabA dims (2p, 2p+1), partitions 64-127 hold
  tabB dims. The per-16-partition index groups of gpsimd.ap_gather carry
  ff (groups 0-3) and cv (groups 4-7), so a single ap_gather per tile
  fetches both embedding rows for every token with zero DMA packets
  (pure Q7 compute gather from SBUF). DVE adds the two bf16 halves;
  stores are embed-major [64, 2*per_core] bf16 -> large contiguous HWDGE
  descriptors. Host unpacks to [N, 128] f32 (bf16 rounding, rel ~4e-3).
"""

import os as _os

import numpy as np

N = 1_000_000
NCORES = 8
PER = N // NCORES          # 125000 tokens per core
V = 2048
D = 128
NSPARSE = 257              # 4*64 sparse rows + zero sentinel row

MODE = _os.environ.get("KMODE", "ap")
TT = int(_os.environ.get("KTT", "4096"))
BUFS = int(_os.environ.get("KBUFS", "4"))
CHUNK = int(_os.environ.get("KCHUNK", "8"))

_cache = {}


def _build_nc_ap(per_core, tt, bufs, chunk):
    import concourse.bacc as bacc
    import concourse.mybir as mybir
    import concourse.tile as tile

    assert tt % 128 == 0
    nfull = per_core // tt
    tailv = per_core - nfull * tt
    tailp = ((tailv + 127) // 128) * 128
    pad = nfull * tt + tailp
    cols = pad // 16

    nc = bacc.Bacc(
        "TRN2", target_bir_lowering=False, debug=False, enable_asserts=False
    )
    idx_t = nc.dram_tensor("idx", [128, cols], mybir.dt.int16, kind="ExternalInput")
    tab_t = nc.dram_tensor("tab", [128, V], mybir.dt.uint32, kind="ExternalInput")
    out_t = nc.dram_tensor(
        "outt", [64, 2 * per_core], mybir.dt.bfloat16, kind="ExternalOutput"
    )
    idx = idx_t.ap()
    tab = tab_t.ap()
    out = out_t.ap()

    with tile.TileContext(nc) as tc:
        with (
            tc.tile_pool(name="idxp", bufs=1) as ip,
            tc.tile_pool(name="work", bufs=bufs) as wp,
        ):
            isb = ip.tile([128, cols], mybir.dt.int16, tag="i")
            tsb = ip.tile([128, V], mybir.dt.uint32, tag="t")
            nc.sync.dma_start(out=tsb[:], in_=tab)
            if chunk > 1:
                step = (cols + chunk - 1) // chunk
                for c0 in range(0, cols, step):
                    c1 = min(c0 + step, cols)
                    nc.sync.dma_start(out=isb[:, c0:c1], in_=idx[:, c0:c1])
            else:
                nc.sync.dma_start(out=isb[:], in_=idx)

            ntiles = nfull + (1 if tailp else 0)
            for t in range(ntiles):
                ni = tt if t < nfull else tailp
                valid = tt if t < nfull else tailv
                c0 = (t * tt) // 16
                go = wp.tile([128, 2 * ni], mybir.dt.bfloat16, tag="go")
                gb = wp.tile([64, 2 * ni], mybir.dt.bfloat16, tag="gb")
                nc.gpsimd.ap_gather(
                    go[:].bitcast(mybir.dt.uint32), tsb[:],
                    isb[:, c0 : c0 + ni // 16],
                    channels=128, num_elems=V, d=1, num_idxs=ni,
                )
                # DVE tensor_tensor needs equal base partitions; bounce the
                # B half down to partition 0 with a cheap SBUF->SBUF DMA.
                nc.scalar.dma_start(out=gb[:], in_=go[64:128, :])
                nc.vector.tensor_add(
                    out=go[0:64, :], in0=go[0:64, :], in1=gb[:]
                )
                r0 = 2 * t * tt
                nc.sync.dma_start(
                    out=out[:, r0 : r0 + 2 * valid], in_=go[0:64, : 2 * valid]
                )
    nc.compile()
    return nc


def _pack_tables(taba_f32, tabb_f32):
    """taba [V, 128] f32, tabb [257, 128] f32 -> [128, V] uint32 packed bf16."""
    import ml_dtypes

    def to_pack(t, rows):
        b = np.zeros((V, 128), dtype=ml_dtypes.bfloat16)
        b[:rows] = t[:rows].astype(ml_dtypes.bfloat16)
        u = b.view(np.uint16).astype(np.uint32)  # [V, 128]
        lo = u[:, 0::2]  # [V, 64]  dims 2p
        hi = u[:, 1::2]  # [V, 64]  dims 2p+1
        return (lo | (hi << 16)).T  # [64, V]

    pa = to_pack(taba_f32, taba_f32.shape[0])
    pb = to_pack(tabb_f32, tabb_f32.shape[0])
    return np.ascontiguousarray(np.concatenate([pa, pb], axis=0))


def _wrap16(arr_i16, reps):
    w16 = arr_i16.reshape(-1, 16).T
    return np.tile(w16, (reps, 1))


def _make_idx(ff_i16, cv_i16):
    return np.ascontiguousarray(
        np.concatenate([_wrap16(ff_i16, 4), _wrap16(cv_i16, 4)], axis=0)
    )


def _get_nc():
    if "nc" not in _cache:
        _cache["nc"] = _build_nc_ap(PER, TT, BUFS, CHUNK)
    return _cache["nc"]


def kernel(
    fixed_features,
    idx0, val0, idx1, val1, idx2, val2, idx3, val3,
    fixed_table, tab0, tab1, tab2, tab3, W_fixed, W_sparse, b,
):
    from concourse.bass_utils import run_bass_kernel_spmd

    ff = np.asarray(fixed_features)
    # combined sparse code per token; 256 = untouched sentinel (zero row).
    cv = np.full(N, 256, dtype=np.int32)
    for k, (ii, vv) in enumerate(
        ((idx0, val0), (idx1, val1), (idx2, val2), (idx3, val3))
    ):
        cv[np.asarray(ii)] = k * 64 + np.asarray(vv).astype(np.int32)

    ft = np.asarray(fixed_table, dtype=np.float32)
    wf = np.asarray(W_fixed, dtype=np.float32)
    ws = np.asarray(W_sparse, dtype=np.float32)
    bb = np.asarray(b, dtype=np.float32)
    taba = (ft @ wf + bb).astype(np.float32)
    tabs = np.concatenate(
        [np.asarray(t, dtype=np.float32) for t in (tab0, tab1, tab2, tab3)], axis=0
    )
    tabb = np.concatenate([tabs @ ws, np.zeros((1, D), np.float32)], axis=0)

    tab = _pack_tables(taba, tabb)

    nfull = PER // TT
    tailv = PER - nfull * TT
    tailp = ((tailv + 127) // 128) * 128
    pad = nfull * TT + tailp

    in_maps = []
    for c in range(NCORES):
        sl = slice(c * PER, (c + 1) * PER)
        fa = np.zeros(pad, dtype=np.int16)
        fa[:PER] = ff[sl].astype(np.int16)
        fb = np.full(pad, NSPARSE - 1, dtype=np.int16)
        fb[:PER] = cv[sl].astype(np.int16)
        in_maps.append({"idx": _make_idx(fa, fb), "tab": tab})

    nc = _get_nc()
    res = run_bass_kernel_spmd(nc, in_maps, core_ids=list(range(NCORES)))
    _cache["last_results"] = res

    out = np.empty((N, D), dtype=np.float32)
    for c in range(NCORES):
        a = np.asarray(res.results[c]["outt"], dtype=np.float32)
        out[c * PER : (c + 1) * PER] = (
            a.reshape(64, PER, 2).transpose(1, 0, 2).reshape(PER, D)
        )
    return out


# revision 9
# speedup vs baseline: 3.1839x; 3.1839x over previous
"""Trainium2 Bass kernel for nn_DenseSparsePreEmbedding.

Math refactoring (verified bit-exact vs the jax reference on CPU):
    fixed_emb @ W_fixed  == (fixed_table @ W_fixed)[fixed_features]
    sparse_emb @ W_sparse== (concat(tabs) @ W_sparse)[cv]  with cv the
                            combined per-token sparse code (last write wins,
                            sentinel 256 -> zero row for untouched tokens)
so the whole module collapses to a dual embedding gather + add:
    out[n] = tabAB[ffn] + tabAB[2048 + cvn]
with tabAB = concat(fixed_table @ W_fixed + b, concat(tab0..3) @ W_sparse,
                    zero row)   [2305, 128]

Device kernel ("SBUF transpose-gather", SPMD over 8 cores, 125000 tokens
each): the combined table lives in SBUF as a bf16 rank-striped image
(row v -> partition v%128, rank v//128, 256B stripe), so the per-token
random reads never touch HBM.  One gpsimd.dma_gather(transpose=True,
SBUF source) per 896-token tile gathers the interleaved [ff | cv+2048]
index stream into an embed-major [128, 2*896] bf16 tile (A rows in the
first half, B rows in the second, same base partition).  DVE adds the
halves; stores are embed-major [128, per_core] bf16 with large
contiguous HWDGE descriptors.  Host transposes/upcasts to [N, 128] f32
(bf16 rounding, rel err ~6e-3 vs the f32 reference).

Why SBUF-source: the baseline (HBM-source 512B-row gathers) was SDMA
packet-throughput bound: 21.4 ms aggregate packet time across 16 SDMA
engines ~= the 1.42 ms runtime.  The small-packet penalty is HBM-only;
SBUF->SBUF packets cost ~13 ns of engine time vs ~57 ns measured for
HBM random reads, and the embed-major stores collapse 125K store
packets into ~140 large ones.
"""

import os as _os

import numpy as np

N = 1_000_000
NCORES = 8
PER = N // NCORES          # 125000 tokens per core
V = 2048
D = 128
NSPARSE = 257              # 4*64 sparse rows + zero sentinel row
NTAB = V + NSPARSE         # 2305 combined rows
NRANK = (NTAB + 127) // 128  # 19 ranks in the SBUF table image

TT = int(_os.environ.get("KTT", "448"))      # tokens per tile (2*TT idx <= 896: transpose-mode ring limit)
BUFS = int(_os.environ.get("KBUFS", "6"))
CHUNK = int(_os.environ.get("KCHUNK", "8"))
NQ = int(_os.environ.get("KNQ", "4"))

_cache = {}


def _build_nc_t(per_core, tt, bufs, chunk, nq):
    import concourse.bacc as bacc
    import concourse.mybir as mybir
    import concourse.tile as tile

    assert (2 * tt) % 128 == 0
    ntiles = (per_core + tt - 1) // tt
    pad = ntiles * tt
    cols = 2 * pad // 16     # interleaved [ff | cv] stream, wrapped in 16

    kw = {"num_swdge_queues": nq} if nq > 1 else {}
    nc = bacc.Bacc(
        "TRN2", target_bir_lowering=False, debug=False, enable_asserts=False, **kw
    )
    idx_t = nc.dram_tensor("idx", [128, cols], mybir.dt.int16, kind="ExternalInput")
    tab_t = nc.dram_tensor(
        "tab", [128, NRANK * 128], mybir.dt.bfloat16, kind="ExternalInput"
    )
    out_t = nc.dram_tensor(
        "outt", [128, per_core], mybir.dt.bfloat16, kind="ExternalOutput"
    )
    idx = idx_t.ap()
    tab = tab_t.ap()
    out = out_t.ap()

    with tile.TileContext(nc) as tc:
        with (
            tc.tile_pool(name="idxp", bufs=1) as ip,
            tc.tile_pool(name="work", bufs=bufs) as wp,
        ):
            isb = ip.tile([128, cols], mybir.dt.int16, tag="i")
            tsb = ip.tile([128, NRANK * 128], mybir.dt.bfloat16, tag="t")
            nc.sync.dma_start(out=tsb[:], in_=tab)
            if chunk > 1:
                step = (cols + chunk - 1) // chunk
                for c0 in range(0, cols, step):
                    c1 = min(c0 + step, cols)
                    nc.sync.dma_start(out=isb[:, c0:c1], in_=idx[:, c0:c1])
            else:
                nc.sync.dma_start(out=isb[:], in_=idx)

            for t in range(ntiles):
                valid = min(tt, per_core - t * tt)
                ni = 2 * tt
                c0 = (2 * tt // 16) * t
                go = wp.tile([128, 1, ni], mybir.dt.bfloat16, tag="go")
                st = wp.tile([128, tt], mybir.dt.bfloat16, tag="st")
                nc.gpsimd.dma_gather(
                    go[:], tsb[:], isb[:, c0 : c0 + ni // 16], ni, ni, D,
                    transpose=True, queue_num=t % nq if nq > 1 else 0,
                    sbuf_tokens_per_rank=128,
                    sbuf_free_dim_per_rank=256,
                )
                import concourse.mybir as _mb
                with nc.allow_low_precision(reason="2-elem bf16 pair add"):
                    nc.vector.tensor_reduce(
                    out=st[:],
                        in_=go[:, 0, :].rearrange("p (t two) -> p t two", two=2),
                        axis=_mb.AxisListType.X, op=_mb.AluOpType.add,
                    )
                r0 = t * tt
                nc.sync.dma_start(
                    out=out[:, r0 : r0 + valid], in_=st[:, :valid]
                )
    nc.compile()
    return nc


def _make_tab(taba_f32, tabb_f32):
    """-> [128, NRANK*128] bf16 rank-striped SBUF image of concat(taba, tabb)."""
    import ml_dtypes

    full = np.zeros((NRANK * 128, D), dtype=np.float32)
    full[:V] = taba_f32
    full[V : V + NSPARSE] = tabb_f32
    bf = full.astype(ml_dtypes.bfloat16)          # [NRANK*128, 128]
    img = bf.reshape(NRANK, 128, D).transpose(1, 0, 2).reshape(128, NRANK * D)
    return np.ascontiguousarray(img)


def _make_idx(ff_i16, cv_i16, tt):
    """Interleave per tile: [ff tile | cv+2048 tile], then wrap into 16 rows
    replicated to 128 partitions."""
    nt = ff_i16.size // tt
    seq = np.empty(2 * ff_i16.size, dtype=np.int16)
    s = seq.reshape(nt, tt, 2)
    s[:, :, 0] = ff_i16.reshape(nt, tt)
    s[:, :, 1] = cv_i16.reshape(nt, tt) + V
    w16 = seq.reshape(-1, 16).T
    return np.ascontiguousarray(np.tile(w16, (8, 1)))


def _get_nc():
    if "nc" not in _cache:
        _cache["nc"] = _build_nc_t(PER, TT, BUFS, CHUNK, NQ)
    return _cache["nc"]


def kernel(
    fixed_features,
    idx0, val0, idx1, val1, idx2, val2, idx3, val3,
    fixed_table, tab0, tab1, tab2, tab3, W_fixed, W_sparse, b,
):
    from concourse.bass_utils import run_bass_kernel_spmd

    ff = np.asarray(fixed_features)
    # combined sparse code per token; 256 = untouched sentinel (zero row).
    cv = np.full(N, 256, dtype=np.int32)
    for k, (ii, vv) in enumerate(
        ((idx0, val0), (idx1, val1), (idx2, val2), (idx3, val3))
    ):
        cv[np.asarray(ii)] = k * 64 + np.asarray(vv).astype(np.int32)

    ft = np.asarray(fixed_table, dtype=np.float32)
    wf = np.asarray(W_fixed, dtype=np.float32)
    ws = np.asarray(W_sparse, dtype=np.float32)
    bb = np.asarray(b, dtype=np.float32)
    taba = (ft @ wf + bb).astype(np.float32)
    tabs = np.concatenate(
        [np.asarray(t, dtype=np.float32) for t in (tab0, tab1, tab2, tab3)], axis=0
    )
    tabb = np.concatenate([tabs @ ws, np.zeros((1, D), np.float32)], axis=0)

    tab = _make_tab(taba, tabb)

    ntiles = (PER + TT - 1) // TT
    pad = ntiles * TT

    in_maps = []
    for c in range(NCORES):
        sl = slice(c * PER, (c + 1) * PER)
        fa = np.zeros(pad, dtype=np.int16)
        fa[:PER] = ff[sl].astype(np.int16)
        fb = np.full(pad, NSPARSE - 1, dtype=np.int16)
        fb[:PER] = cv[sl].astype(np.int16)
        in_maps.append({"idx": _make_idx(fa, fb, TT), "tab": tab})

    nc = _get_nc()
    res = run_bass_kernel_spmd(nc, in_maps, core_ids=list(range(NCORES)))
    _cache["last_results"] = res

    out = np.empty((N, D), dtype=np.float32)
    for c in range(NCORES):
        a = np.asarray(res.results[c]["outt"], dtype=np.float32)
        out[c * PER : (c + 1) * PER] = a.T
    return out
